# revision 8
# baseline (speedup 1.0000x reference)
"""NF4-quantized linear layer (x @ dequant(W).T + dequant(b)) on 8 Trainium2 cores.

Strategy (column-parallel / tensor-parallel):
  - Shard the out_features dim (14336) into 8 shards of 1792; replicate x.
  - Host side: relabel the packed 4-bit codes through the 16-entry NF4 table
    (pure layout/marshalling: int32-byte -> two bf16 table values) and
    pre-transpose into W.T layout; pre-transpose x into x.T tiles.
  - Device side (per core): apply the per-64-block absmax scaling (DVE),
    run the tiled bf16 matmul with fp32 PSUM accumulation (PE), add bias,
    stream results out.  Weights stay fully resident in SBUF.
  - Gather: concatenate the 8 output shards on the feature axis.
"""

import sys

sys.path.insert(0, "/opt/trn_rl_repo")

import numpy as np
import ml_dtypes

import concourse.bass as bass
import concourse.tile as tile
from concourse import mybir
from concourse.vector_clock import ScopedClock
from concourse.bass_utils import run_bass_kernel_spmd

BF16 = ml_dtypes.bfloat16

OUT_F = 14336
IN_F = 4096
M_ROWS = 8192
BLOCK = 64
N_CORES = 8
SHARD = OUT_F // N_CORES  # 1792

K_TILES = IN_F // 128  # 32
M_TILES = M_ROWS // 128  # 64
N_CHUNKS = [(0, 512), (512, 512), (1024, 512), (1536, 256)]

NF4 = np.array(
    [
        -1.0, -0.6961928009986877, -0.5250730514526367, -0.39491748809814453,
        -0.28444138169288635, -0.18477343022823334, -0.09105003625154495, 0.0,
        0.07958029955625534, 0.16093020141124725, 0.24611230194568634,
        0.33791524171829224, 0.44070982933044434, 0.5626170039176941,
        0.7229568362236023, 1.0,
    ],
    dtype=np.float32,
)


def _patched_drain_and_barrier(self, tick_clock, wait_clock):
    # This walrus build rejects >1 sync-wait on the SP/CTRL-queue drain that
    # Tile emits at kernel tail ("Too many sync wait commands").  Split the
    # waits across extra no-ops, one wait each.
    drain_inst = self.nc.sync.drain()
    wait_clock.add_sem_waits(
        drain_inst.ins, ScopedClock({None: tick_clock.global_clock})
    )
    waits = list(drain_inst.ins.sync_info.on_wait or [])
    if len(waits) > 1:
        drain_inst.ins.sync_info.on_wait = waits[:1]
        for i in range(1, len(waits)):
            nop = self.nc.sync.nop(nofuse=True)
            nop.ins.sync_info = mybir.SyncInfo(on_wait=waits[i : i + 1], on_update=[])
    self.nc.all_engine_barrier()
    assert self.sems is not None
    popped = self.nc._tile_sem_poison_stack.pop()
    assert popped is self._sem_poison
    self.nc.clear_and_free_semaphores(list(self.sems.allocated().values()))
    self.nc.all_engine_barrier()


tile.TileContext._drain_and_barrier = _patched_drain_and_barrier


def _split_multi_waits(nc, max_waits=1):
    """This walrus build accepts at most one sync-wait per instruction.
    Move extra waits onto same-engine no-ops inserted just before the
    instruction (engine queues are in-order, so semantics are unchanged)."""
    n = 0
    for f in nc.m.functions:
        for bb in f.blocks:
            out_list = []
            for ins in bb.instructions:
                si = getattr(ins, "sync_info", None)
                waits = list(si.on_wait) if si is not None and si.on_wait else []
                if len(waits) > max_waits:
                    for w in waits[: len(waits) - max_waits]:
                        nop = mybir.InstNoOp(
                            name=f"I-waitsplit-{n}",
                            ins=[],
                            outs=[],
                            engine=ins.engine,
                            sync_info=mybir.SyncInfo(on_wait=[w], on_update=[]),
                        )
                        n += 1
                        out_list.append(nop)
                    si.on_wait = waits[len(waits) - max_waits :]
                out_list.append(ins)
            bb.instructions[:] = out_list
    return n


def _build_program(m_tiles=M_TILES, split_waits=True):
    nc = bass.Bass("TRN2", target_bir_lowering=False, debug=False, num_devices=1)

    wq = nc.dram_tensor("wq", [IN_F, SHARD], mybir.dt.bfloat16, kind="ExternalInput").ap()
    st = nc.dram_tensor("st", [IN_F // BLOCK, SHARD], mybir.dt.float32, kind="ExternalInput").ap()
    xt = nc.dram_tensor("xt", [m_tiles, 128, K_TILES, 128], mybir.dt.bfloat16, kind="ExternalInput").ap()
    bias = nc.dram_tensor("bias", [SHARD], mybir.dt.float32, kind="ExternalInput").ap()
    out = nc.dram_tensor("out", [m_tiles * 128, SHARD], mybir.dt.float32, kind="ExternalOutput").ap()

    with tile.TileContext(nc) as tc:
        with (
            tc.tile_pool(name="wres", bufs=1) as wres_pool,
            tc.tile_pool(name="bias", bufs=1) as bias_pool,
            tc.tile_pool(name="stage", bufs=3) as stage_pool,
            tc.tile_pool(name="xin", bufs=3) as x_pool,
            tc.tile_pool(name="oput", bufs=6) as o_pool,
            tc.tile_pool(name="psum", bufs=8, space="PSUM") as ps_pool,
        ):
            # Resident scaled weights: W.T layout, k-tile t at cols [t*SHARD, (t+1)*SHARD)
            wsc = wres_pool.tile([128, K_TILES * SHARD], mybir.dt.bfloat16)

            # Bias replicated across partitions (free dim = out features)
            bias_sb = bias_pool.tile([128, SHARD], mybir.dt.float32)
            nc.sync.dma_start(bias_sb[:], bias.partition_broadcast(128))

            # Dequant: per k-tile multiply raw NF4 values by broadcast absmax
            for t in range(K_TILES):
                wq_s = stage_pool.tile([128, SHARD], mybir.dt.bfloat16, tag="wq")
                nc.sync.dma_start(wq_s[:], wq[t * 128 : (t + 1) * 128, :])
                sc_s = stage_pool.tile([128, SHARD], mybir.dt.float32, tag="sc")
                nc.sync.dma_start(
                    sc_s[0:64, :], st[2 * t, :].partition_broadcast(64)
                )
                nc.sync.dma_start(
                    sc_s[64:128, :], st[2 * t + 1, :].partition_broadcast(64)
                )
                nc.vector.tensor_mul(
                    wsc[:, t * SHARD : (t + 1) * SHARD], wq_s[:], sc_s[:]
                )

            # Main matmul loop
            for m in range(m_tiles):
                xts = x_pool.tile([128, IN_F], mybir.dt.bfloat16, tag="xts")
                nc.sync.dma_start(xts[:], xt[m].rearrange("p t j -> p (t j)"))
                for n0, nw in N_CHUNKS:
                    ps = ps_pool.tile([128, 512], mybir.dt.float32, tag="ps")
                    for t in range(K_TILES):
                        nc.tensor.matmul(
                            ps[:, :nw],
                            lhsT=xts[:, t * 128 : (t + 1) * 128],
                            rhs=wsc[:, t * SHARD + n0 : t * SHARD + n0 + nw],
                            start=(t == 0),
                            stop=(t == K_TILES - 1),
                        )
                    ot = o_pool.tile([128, 512], mybir.dt.float32, tag="ot")
                    nc.vector.tensor_add(ot[:, :nw], ps[:, :nw], bias_sb[:, n0 : n0 + nw])
                    nc.sync.dma_start(
                        out[m * 128 : (m + 1) * 128, n0 : n0 + nw], ot[:, :nw]
                    )

    if split_waits:
        _split_multi_waits(nc)
    return nc


_PROGRAM = None


def _get_program():
    global _PROGRAM
    if _PROGRAM is None:
        _PROGRAM = _build_program()
    return _PROGRAM


def _prep_inputs(x, w_packed, w_absmax, b_packed, b_absmax):
    """Host-side marshalling: NF4 code relabeling, layout transposes, sharding."""
    nf4_bf16 = NF4.astype(BF16)

    # Weights: packed int32 bytes -> W.T [IN_F, OUT_F] bf16 of unscaled NF4 values
    b = np.asarray(w_packed).astype(np.uint8).reshape(OUT_F, IN_F // 2)
    bT = np.ascontiguousarray(b.T)  # [2048, 14336]
    valsT = np.empty((IN_F, OUT_F), dtype=BF16)
    valsT[0::2] = nf4_bf16[bT >> 4]
    valsT[1::2] = nf4_bf16[bT & 15]

    # Scales: [OUT_F, 64] -> per-shard [64, SHARD]
    am = np.asarray(w_absmax, dtype=np.float32).reshape(OUT_F, IN_F // BLOCK)

    # x: [M, K] f32 -> bf16 tiles [m_tile, p(k%128), k_tile, j(m%128)]
    xbf = np.asarray(x, dtype=np.float32).astype(BF16)
    xt5 = np.ascontiguousarray(
        xbf.reshape(M_TILES, 128, K_TILES, 128).transpose(0, 3, 2, 1)
    )

    # Bias: full dequant on host (14336 elements — negligible)
    bb = np.asarray(b_packed).astype(np.uint8)
    bcodes = np.empty(OUT_F, dtype=np.uint8)
    bcodes[0::2] = bb >> 4
    bcodes[1::2] = bb & 15
    bias_full = (
        NF4[bcodes].reshape(-1, BLOCK)
        * np.asarray(b_absmax, dtype=np.float32).reshape(-1, 1)
    ).reshape(OUT_F)

    in_maps = []
    for c in range(N_CORES):
        n0, n1 = c * SHARD, (c + 1) * SHARD
        in_maps.append(
            {
                "wq": np.ascontiguousarray(valsT[:, n0:n1]),
                "st": np.ascontiguousarray(am[n0:n1].T),
                "xt": xt5,
                "bias": np.ascontiguousarray(bias_full[n0:n1]),
            }
        )
    return in_maps


def kernel(x, w_packed, w_absmax, b_packed, b_absmax, trace=False, **run_kwargs):
    nc = _get_program()
    in_maps = _prep_inputs(x, w_packed, w_absmax, b_packed, b_absmax)
    res = run_bass_kernel_spmd(
        nc, in_maps, core_ids=list(range(N_CORES)), trace=trace, **run_kwargs
    )
    out = np.concatenate([res.results[c]["out"] for c in range(N_CORES)], axis=1)
    kernel.last_results = res
    return out


# revision 12
# speedup vs baseline: 1.0642x; 1.0642x over previous
"""NF4-quantized linear layer (x @ dequant(W).T + dequant(b)) on 8 Trainium2 cores.

Strategy (column-parallel / tensor-parallel):
  - Shard the out_features dim (14336) into 8 shards of 1792; replicate x.
  - Host side: relabel the packed 4-bit codes through the 16-entry NF4 table
    (pure layout/marshalling: int32-byte -> two bf16 table values) and
    pre-transpose into W.T layout; pre-transpose x into x.T tiles.
  - Device side (per core): apply the per-64-block absmax scaling (DVE),
    run the tiled bf16 matmul with fp32 PSUM accumulation (PE), add bias,
    stream results out.  Weights stay fully resident in SBUF.
  - Gather: concatenate the 8 output shards on the feature axis.
"""

import sys

sys.path.insert(0, "/opt/trn_rl_repo")

import numpy as np
import ml_dtypes

import concourse.bass as bass
import concourse.tile as tile
from concourse import mybir
from concourse.vector_clock import ScopedClock
from concourse.bass_utils import run_bass_kernel_spmd

BF16 = ml_dtypes.bfloat16

OUT_F = 14336
IN_F = 4096
M_ROWS = 8192
BLOCK = 64
N_CORES = 8
SHARD = OUT_F // N_CORES  # 1792

K_TILES = IN_F // 128  # 32
M_TILES = M_ROWS // 128  # 64
N_CHUNKS = [(0, 512), (512, 512), (1024, 512), (1536, 256)]

NF4 = np.array(
    [
        -1.0, -0.6961928009986877, -0.5250730514526367, -0.39491748809814453,
        -0.28444138169288635, -0.18477343022823334, -0.09105003625154495, 0.0,
        0.07958029955625534, 0.16093020141124725, 0.24611230194568634,
        0.33791524171829224, 0.44070982933044434, 0.5626170039176941,
        0.7229568362236023, 1.0,
    ],
    dtype=np.float32,
)


def _patched_drain_and_barrier(self, tick_clock, wait_clock):
    # This walrus build rejects >1 sync-wait on the SP/CTRL-queue drain that
    # Tile emits at kernel tail ("Too many sync wait commands").  Split the
    # waits across extra no-ops, one wait each.
    drain_inst = self.nc.sync.drain()
    wait_clock.add_sem_waits(
        drain_inst.ins, ScopedClock({None: tick_clock.global_clock})
    )
    waits = list(drain_inst.ins.sync_info.on_wait or [])
    if len(waits) > 1:
        drain_inst.ins.sync_info.on_wait = waits[:1]
        for i in range(1, len(waits)):
            nop = self.nc.sync.nop(nofuse=True)
            nop.ins.sync_info = mybir.SyncInfo(on_wait=waits[i : i + 1], on_update=[])
    self.nc.all_engine_barrier()
    assert self.sems is not None
    popped = self.nc._tile_sem_poison_stack.pop()
    assert popped is self._sem_poison
    self.nc.clear_and_free_semaphores(list(self.sems.allocated().values()))
    self.nc.all_engine_barrier()


tile.TileContext._drain_and_barrier = _patched_drain_and_barrier


def _split_multi_waits(nc, max_waits=1):
    """This walrus build accepts at most one sync-wait per instruction.
    Move extra waits onto same-engine no-ops inserted just before the
    instruction (engine queues are in-order, so semantics are unchanged)."""
    n = 0
    for f in nc.m.functions:
        for bb in f.blocks:
            out_list = []
            for ins in bb.instructions:
                si = getattr(ins, "sync_info", None)
                waits = list(si.on_wait) if si is not None and si.on_wait else []
                if len(waits) > max_waits:
                    for w in waits[: len(waits) - max_waits]:
                        nop = mybir.InstNoOp(
                            name=f"I-waitsplit-{n}",
                            ins=[],
                            outs=[],
                            engine=ins.engine,
                            sync_info=mybir.SyncInfo(on_wait=[w], on_update=[]),
                        )
                        n += 1
                        out_list.append(nop)
                    si.on_wait = waits[len(waits) - max_waits :]
                out_list.append(ins)
            bb.instructions[:] = out_list
    return n


def _build_program(m_tiles=M_TILES, split_waits=True):
    nc = bass.Bass("TRN2", target_bir_lowering=False, debug=False, num_devices=1)

    wq = nc.dram_tensor("wq", [IN_F, SHARD], mybir.dt.bfloat16, kind="ExternalInput").ap()
    st = nc.dram_tensor("st", [IN_F // BLOCK, SHARD], mybir.dt.bfloat16, kind="ExternalInput").ap()
    xt = nc.dram_tensor("xt", [m_tiles, 128, K_TILES, 128], mybir.dt.bfloat16, kind="ExternalInput").ap()
    bias = nc.dram_tensor("bias", [SHARD], mybir.dt.float32, kind="ExternalInput").ap()
    out = nc.dram_tensor("out", [m_tiles * 128, SHARD], mybir.dt.float32, kind="ExternalOutput").ap()

    with tile.TileContext(nc) as tc:
        with (
            tc.tile_pool(name="wres", bufs=1) as wres_pool,
            tc.tile_pool(name="bias", bufs=1) as bias_pool,
            tc.tile_pool(name="stage", bufs=3) as stage_pool,
            tc.tile_pool(name="xin", bufs=4) as x_pool,
            tc.tile_pool(name="oput", bufs=6) as o_pool,
            tc.tile_pool(name="psum", bufs=8, space="PSUM") as ps_pool,
        ):
            # Resident scaled weights: W.T layout, k-tile t at cols [t*SHARD, (t+1)*SHARD)
            wsc = wres_pool.tile([128, K_TILES * SHARD], mybir.dt.bfloat16)

            # Bias replicated across partitions (free dim = out features)
            bias_sb = bias_pool.tile([128, SHARD], mybir.dt.float32)
            nc.sync.dma_start(bias_sb[:], bias.partition_broadcast(128))

            # Prefetch the first x slabs on the SP HWDGE ring so the PE can
            # start as soon as k-tile 0 is dequantized; the dequant bulk
            # rides the ACT HWDGE ring instead.
            X_PREFETCH = min(4, m_tiles)
            x_tiles = []
            for m in range(X_PREFETCH):
                xts = x_pool.tile([128, IN_F], mybir.dt.bfloat16, tag="xts", name=f"xts{m}")
                nc.sync.dma_start(xts[:], xt[m].rearrange("p t j -> p (t j)"))
                x_tiles.append(xts)

            # Dequant: per k-tile multiply raw NF4 values by broadcast absmax
            for t in range(K_TILES):
                wq_s = stage_pool.tile([128, SHARD], mybir.dt.bfloat16, tag="wq")
                nc.scalar.dma_start(wq_s[:], wq[t * 128 : (t + 1) * 128, :])
                sc_s = stage_pool.tile([128, SHARD], mybir.dt.bfloat16, tag="sc")
                nc.scalar.dma_start(
                    sc_s[0:64, :], st[2 * t, :].partition_broadcast(64)
                )
                nc.scalar.dma_start(
                    sc_s[64:128, :], st[2 * t + 1, :].partition_broadcast(64)
                )
                nc.vector.tensor_mul(
                    wsc[:, t * SHARD : (t + 1) * SHARD], wq_s[:], sc_s[:]
                )

            # Main matmul loop
            for m in range(m_tiles):
                if m < X_PREFETCH:
                    xts = x_tiles[m]
                else:
                    xts = x_pool.tile([128, IN_F], mybir.dt.bfloat16, tag="xts", name=f"xts{m}")
                    nc.sync.dma_start(xts[:], xt[m].rearrange("p t j -> p (t j)"))
                for n0, nw in N_CHUNKS:
                    ps = ps_pool.tile([128, 512], mybir.dt.float32, tag="ps")
                    for t in range(K_TILES):
                        nc.tensor.matmul(
                            ps[:, :nw],
                            lhsT=xts[:, t * 128 : (t + 1) * 128],
                            rhs=wsc[:, t * SHARD + n0 : t * SHARD + n0 + nw],
                            start=(t == 0),
                            stop=(t == K_TILES - 1),
                        )
                    ot = o_pool.tile([128, 512], mybir.dt.float32, tag="ot")
                    nc.vector.tensor_add(ot[:, :nw], ps[:, :nw], bias_sb[:, n0 : n0 + nw])
                    nc.sync.dma_start(
                        out[m * 128 : (m + 1) * 128, n0 : n0 + nw], ot[:, :nw]
                    )

    if split_waits:
        _split_multi_waits(nc)
    return nc


_PROGRAM = None


def _get_program():
    global _PROGRAM
    if _PROGRAM is None:
        _PROGRAM = _build_program()
    return _PROGRAM


def _prep_inputs(x, w_packed, w_absmax, b_packed, b_absmax):
    """Host-side marshalling: NF4 code relabeling, layout transposes, sharding."""
    nf4_bf16 = NF4.astype(BF16)

    # Weights: packed int32 bytes -> W.T [IN_F, OUT_F] bf16 of unscaled NF4 values
    b = np.asarray(w_packed).astype(np.uint8).reshape(OUT_F, IN_F // 2)
    bT = np.ascontiguousarray(b.T)  # [2048, 14336]
    valsT = np.empty((IN_F, OUT_F), dtype=BF16)
    valsT[0::2] = nf4_bf16[bT >> 4]
    valsT[1::2] = nf4_bf16[bT & 15]

    # Scales: [OUT_F, 64] -> per-shard [64, SHARD]
    am = np.asarray(w_absmax, dtype=np.float32).reshape(OUT_F, IN_F // BLOCK)

    # x: [M, K] f32 -> bf16 tiles [m_tile, p(k%128), k_tile, j(m%128)]
    xbf = np.asarray(x, dtype=np.float32).astype(BF16)
    xt5 = np.ascontiguousarray(
        xbf.reshape(M_TILES, 128, K_TILES, 128).transpose(0, 3, 2, 1)
    )

    # Bias: full dequant on host (14336 elements — negligible)
    bb = np.asarray(b_packed).astype(np.uint8)
    bcodes = np.empty(OUT_F, dtype=np.uint8)
    bcodes[0::2] = bb >> 4
    bcodes[1::2] = bb & 15
    bias_full = (
        NF4[bcodes].reshape(-1, BLOCK)
        * np.asarray(b_absmax, dtype=np.float32).reshape(-1, 1)
    ).reshape(OUT_F)

    in_maps = []
    for c in range(N_CORES):
        n0, n1 = c * SHARD, (c + 1) * SHARD
        in_maps.append(
            {
                "wq": np.ascontiguousarray(valsT[:, n0:n1]),
                "st": np.ascontiguousarray(am[n0:n1].T).astype(BF16),
                "xt": xt5,
                "bias": np.ascontiguousarray(bias_full[n0:n1]),
            }
        )
    return in_maps


def kernel(x, w_packed, w_absmax, b_packed, b_absmax, trace=False, **run_kwargs):
    nc = _get_program()
    in_maps = _prep_inputs(x, w_packed, w_absmax, b_packed, b_absmax)
    res = run_bass_kernel_spmd(
        nc, in_maps, core_ids=list(range(N_CORES)), trace=trace, **run_kwargs
    )
    out = np.concatenate([res.results[c]["out"] for c in range(N_CORES)], axis=1)
    kernel.last_results = res
    return out
